# revision 1
# baseline (speedup 1.0000x reference)
"""Trainium2 Bass kernel for nn_CausalAttentionSortNet.

Math (per bh slice, reformulated as constant matmuls):
  sq[i, d] = (1/8) * (1/(64*i+1)) * sum_{t<=64*i} q[t, d]          = Aq @ q
  sk[j, d] = sum_{t in bucket j} cumsum(k)[t]/(t+1) summed weights  = Mk @ k
  Rc[i, j] = sum_d sq[i,d]*sk[j,d]                (= R[:, 1:], col 0 of R is 0)
  R masked where (col-1) >= row, then hard top-1 of softmax:
  out[i, jmax] = 1/sum_j exp(R[i,j]-max_j R), zero elsewhere.

Both Aq [64,4096] and Mk [64,4096] are data-independent, so the heavy part is
two streaming matmuls over q and k per bh (memory-bound). Sharding: bh axis
across 8 cores, 8 bh per core, zero communication.

On-chip layout per core: data tiles [128p, 2bh, 32r, 64d] with t = 32*p + r
(fully contiguous 1MB-per-bh DMAs; q on the SP HWDGE ring, k on the ACT ring).
Matmul (per bh half b, per chunk r): stationary lhsT = data[:, b, r, :]
(2D [K=128, M=64] — walrus requires one free dim on the stationary AP),
moving rhs = W[:, r, :] (N=64 summary rows), accumulated into PSUM
[128, 64] = [(b,d), i] at partition offset 64*b. All PE/vector/scalar work
hides under the input DMA stream (~44 us/core = ~381 GB/s, HBM roofline).
"""

import numpy as np

BH, SEQ, DIM = 64, 4096, 64
NCORES = 8
BH_PER_CORE = BH // NCORES
GROUPS = BH_PER_CORE // 2  # 2 bh per group
FLTMAX = float(np.finfo(np.float32).max)

_CACHE = {}


def _constants():
    t = np.arange(SEQ, dtype=np.float64)
    i = np.arange(64, dtype=np.float64)[:, None]
    # Aq[i, t] = 1/(8*(64i+1)) for t <= 64i else 0   (includes the dim^-0.5 = 1/8)
    aq = np.where(t[None, :] <= 64 * i, 1.0 / (8.0 * (64 * i + 1.0)), 0.0)
    # Mk[j, t]: weight of k[t] in sk[j] = sum over bucket-j of cumavg
    inv = 1.0 / (t + 1.0)
    invb = inv.reshape(64, 64)
    suffix = np.cumsum(invb[:, ::-1], axis=1)[:, ::-1]  # suffix[j, s] = sum_{u>=s} 1/(64j+u+1)
    cj = invb.sum(axis=1)
    mk = np.zeros((64, SEQ))
    for j in range(64):
        mk[j, : 64 * j] = cj[j]
        mk[j, 64 * j : 64 * j + 64] = suffix[j]
    # SBUF weight layout [p, r, i] with t = 32p + r
    wq = aq.T.reshape(128, 32, 64).astype(np.float32)
    wk = mk.T.reshape(128, 32, 64).astype(np.float32)
    wq = np.ascontiguousarray(wq)
    wk = np.ascontiguousarray(wk)
    # additive causal mask on R[:, 1:]: masked where jc >= i
    maskadd = np.where(
        np.arange(64)[None, :] >= np.arange(64)[:, None], -FLTMAX, 0.0
    ).astype(np.float32)
    return wq, wk, maskadd


def _build_nc(reps=1, dma_only=False, variant=0):
    """variant 0: all input DMAs on the SP HWDGE ring, 2MB each, bufs=2.
    variant 1: q on SP ring / k on ACT ring, per-bh 1MB DMAs, bufs=3.
    variant 2: variant 1 + skip q rows t in [4064, 4096) (partition 127):
      they are never used (sq[63] needs only t<=4032) and their Aq weight
      rows are zero, so q DMAs load 127 partitions and q matmuls contract
      K=127 — bit-identical output, 0.38% fewer HBM bytes."""
    from contextlib import ExitStack

    import concourse.bacc as bacc
    import concourse.mybir as mybir
    import concourse.tile as tile

    f32 = mybir.dt.float32
    wq_np, wk_np, mask_np = _constants()

    nc = bacc.Bacc(trn_type="TRN2")
    q = nc.dram_tensor("q", [BH_PER_CORE, SEQ, DIM], f32, kind="ExternalInput")
    k = nc.dram_tensor("k", [BH_PER_CORE, SEQ, DIM], f32, kind="ExternalInput")
    out = nc.dram_tensor("out", [BH_PER_CORE, 64, 65], f32, kind="ExternalOutput")
    wq_dram = nc.inline_tensor(wq_np, "wq_const")
    wk_dram = nc.inline_tensor(wk_np, "wk_const")
    mask_dram = nc.inline_tensor(mask_np, "mask_const")

    q_ap, k_ap, out_ap = q.ap(), k.ap(), out.ap()

    with tile.TileContext(nc) as tc, ExitStack() as ctx:
        singles = ctx.enter_context(tc.tile_pool(name="singles", bufs=1))
        data = ctx.enter_context(tc.tile_pool(name="data", bufs=3 if variant else 2))
        small = ctx.enter_context(tc.tile_pool(name="small", bufs=3))
        psum = ctx.enter_context(tc.tile_pool(name="psum", bufs=2, space="PSUM"))
        rpsum = ctx.enter_context(tc.tile_pool(name="rpsum", bufs=2, space="PSUM"))

        # Constants go on the SWDGE (gpsimd) queue so they don't serialize
        # ahead of the first data loads on the two HWDGE rings.
        wq_sb = singles.tile([128, 32, 64], f32)
        wk_sb = singles.tile([128, 32, 64], f32)
        mask_sb = singles.tile([64, 64], f32)
        nc.gpsimd.dma_start(wq_sb[:], wq_dram.ap())
        nc.gpsimd.dma_start(wk_sb[:], wk_dram.ap())
        nc.gpsimd.dma_start(mask_sb[:], mask_dram.ap())

        for rep_g in range(reps * GROUPS):
            g = rep_g % GROUPS
            qt = data.tile([128, 2, 32, 64], f32, tag="qt")
            kt = data.tile([128, 2, 32, 64], f32, tag="kt")
            if variant:
                qp = 127 if variant >= 2 else 128  # q partitions loaded/contracted
                for b in range(2):
                    nc.sync.dma_start(
                        qt[:qp, b],
                        q_ap[2 * g + b][: qp * 32].rearrange(
                            "(p r) d -> p r d", p=qp
                        ),
                    )
                    nc.scalar.dma_start(
                        kt[:, b],
                        k_ap[2 * g + b].rearrange("(p r) d -> p r d", p=128),
                    )
            else:
                nc.sync.dma_start(
                    qt[:],
                    q_ap[2 * g : 2 * g + 2].rearrange("b (p r) d -> p b r d", p=128),
                )
                nc.sync.dma_start(
                    kt[:],
                    k_ap[2 * g : 2 * g + 2].rearrange("b (p r) d -> p b r d", p=128),
                )
            if dma_only:
                continue
            psq = psum.tile([128, 64], f32, tag="psq")
            psk = psum.tile([128, 64], f32, tag="psk")
            # Stationary (weights) APs must be 2D [K, M] for walrus, so one
            # matmul per bh half: out partitions 64b..64b+64 of the PSUM tile.
            qp = 127 if variant >= 2 else 128
            for b in range(2):
                for r in range(32):
                    nc.tensor.matmul(
                        psq[64 * b : 64 * b + 64, :],
                        lhsT=qt[:qp, b, r, :], rhs=wq_sb[:qp, r, :],
                        start=(r == 0), stop=(r == 31),
                    )
            for b in range(2):
                for r in range(32):
                    nc.tensor.matmul(
                        psk[64 * b : 64 * b + 64, :],
                        lhsT=kt[:, b, r, :], rhs=wk_sb[:, r, :],
                        start=(r == 0), stop=(r == 31),
                    )
            sq_sb = small.tile([128, 64], f32, tag="sq")
            sk_sb = small.tile([128, 64], f32, tag="sk")
            nc.vector.tensor_copy(sq_sb[:], psq[:])
            nc.vector.tensor_copy(sk_sb[:], psk[:])
            for b in range(2):
                bh = 2 * g + b
                pr = rpsum.tile([64, 64], f32, tag="pr")
                nc.tensor.matmul(
                    pr[:],
                    lhsT=sq_sb[64 * b : 64 * b + 64, :],
                    rhs=sk_sb[64 * b : 64 * b + 64, :],
                    start=True, stop=True,
                )
                rf = small.tile([64, 65], f32, tag="rf")
                nc.vector.memset(rf[:, 0:1], 0.0)
                nc.vector.tensor_add(rf[:, 1:65], pr[:], mask_sb[:])
                m = small.tile([64, 1], f32, tag="m")
                nm = small.tile([64, 1], f32, tag="nm")
                s = small.tile([64, 1], f32, tag="s")
                rr = small.tile([64, 1], f32, tag="rr")
                nc.vector.reduce_max(m[:], rf[:], axis=mybir.AxisListType.X)
                nc.vector.tensor_scalar_mul(nm[:], m[:], -1.0)
                e = small.tile([64, 65], f32, tag="e")
                nc.scalar.activation(
                    e[:], rf[:], mybir.ActivationFunctionType.Exp,
                    bias=nm[:], scale=1.0, accum_out=s[:],
                )
                nc.vector.reciprocal(rr[:], s[:])
                o = small.tile([64, 65], f32, tag="o")
                nc.vector.tensor_scalar(
                    out=o[:], in0=rf[:], scalar1=m[:], scalar2=rr[:],
                    op0=mybir.AluOpType.is_equal, op1=mybir.AluOpType.mult,
                )
                nc.sync.dma_start(out_ap[bh], o[:])

    nc.compile()
    nc._kern_key = (reps, dma_only, variant)
    return nc


def _get_nc(reps=1, dma_only=False, variant=0):
    key = ("nc", reps, dma_only, variant)
    if key not in _CACHE:
        _CACHE[key] = _build_nc(reps, dma_only, variant)
    return _CACHE[key]


def _make_runner(nc):
    """Persistent jit(shard_map) callable over the 8 cores for one Bass module.

    One function object per nc so jax.jit's cache is reused across calls
    (run_bass_kernel_spmd re-traces on every invocation)."""
    import jax
    from jax.sharding import Mesh, PartitionSpec
    from jax.experimental.shard_map import shard_map

    import concourse.mybir as mybir
    from concourse.bass2jax import (
        _bass_exec_p,
        install_neuronx_cc_hook,
        partition_id_tensor,
    )

    install_neuronx_cc_hook()

    partition_name = nc.partition_id_tensor.name if nc.partition_id_tensor else None
    in_names, out_names, out_avals, zero_shapes = [], [], [], []
    for alloc in nc.m.functions[0].allocations:
        if not isinstance(alloc, mybir.MemoryLocationSet):
            continue
        name = alloc.memorylocations[0].name
        if alloc.kind == "ExternalInput":
            if name != partition_name:
                in_names.append(name)
        elif alloc.kind == "ExternalOutput":
            out_names.append(name)
            shape = tuple(alloc.tensor_shape)
            dtype = mybir.dt.np(alloc.dtype)
            out_avals.append(jax.core.ShapedArray(shape, dtype))
            zero_shapes.append((shape, dtype))
    n_params = len(in_names)
    n_outs = len(out_avals)
    all_in_names = tuple(
        in_names + out_names + ([partition_name] if partition_name else [])
    )

    def _body(*args):
        operands = list(args)
        if partition_name is not None:
            operands.append(partition_id_tensor())
        return tuple(
            _bass_exec_p.bind(
                *operands,
                out_avals=tuple(out_avals),
                in_names=all_in_names,
                out_names=tuple(out_names),
                lowering_input_output_aliases=(),
                sim_require_finite=True,
                sim_require_nnan=True,
                nc=nc,
            )
        )

    devices = jax.devices()[:NCORES]
    mesh = Mesh(np.asarray(devices), ("core",))
    _CACHE[("runner_mesh",) + getattr(nc, "_kern_key", (1, False, 0))] = mesh
    fn = jax.jit(
        shard_map(
            _body,
            mesh=mesh,
            in_specs=(PartitionSpec("core"),) * (n_params + n_outs),
            out_specs=(PartitionSpec("core"),) * n_outs,
            check_rep=False,
        ),
        donate_argnums=tuple(range(n_params, n_params + n_outs)),
        keep_unused=True,
    )

    name_to_idx = {n: i for i, n in enumerate(in_names)}
    out_idx = out_names.index("out")

    def run(q, k):
        import jax as _jax

        ins = [None] * n_params
        ins[name_to_idx["q"]] = q
        ins[name_to_idx["k"]] = k
        zeros = [
            np.zeros((NCORES * s[0], *s[1:]), dt) for (s, dt) in zero_shapes
        ]
        outs = fn(*ins, *zeros)
        _jax.block_until_ready(outs)
        return np.asarray(outs[out_idx]).reshape(BH, 64, 65)

    return run


def _get_runner(reps=1, dma_only=False, variant=0):
    key = ("runner", reps, dma_only, variant)
    if key not in _CACHE:
        _CACHE[key] = _make_runner(_get_nc(reps, dma_only, variant))
    return _CACHE[key]


def _prep(q, k):
    q = np.ascontiguousarray(np.asarray(q), dtype=np.float32)
    k = np.ascontiguousarray(np.asarray(k), dtype=np.float32)
    return q, k


# Default shipped configuration: dual HWDGE rings (q on SP, k on ACT),
# per-bh 1MB DMAs, triple-buffered data tiles. Variant 2 (skip the unused
# q tail, -0.38% bytes) is numerically correct but measured 5.6x SLOWER in
# a matched-baseline same-window A/B: its 127-partition q DMAs (127 is
# prime) defeat the 16-engine descriptor swizzle and fragment the
# transfer. Do not enable it.
DEFAULT_VARIANT = 1


def _run_spmd(q, k, trace=False, **kwargs):
    q, k = _prep(q, k)
    out = _get_runner(1, variant=DEFAULT_VARIANT)(q, k)
    return out, None


def kernel(q, k, topk=1):
    q, k = _prep(q, k)
    return _get_runner(1, variant=DEFAULT_VARIANT)(q, k)



# revision 8
# speedup vs baseline: 3.7648x; 3.7648x over previous
"""Trainium2 Bass kernel for nn_CausalAttentionSortNet.

Math (per bh slice, reformulated as constant matmuls):
  sq[i, d] = (1/8) * (1/(64*i+1)) * sum_{t<=64*i} q[t, d]          = Aq @ q
  sk[j, d] = sum_{t in bucket j} cumsum(k)[t]/(t+1) summed weights  = Mk @ k
  Rc[i, j] = sum_d sq[i,d]*sk[j,d]                (= R[:, 1:], col 0 of R is 0)
  R masked where (col-1) >= row, then hard top-1 of softmax:
  out[i, jmax] = 1/sum_j exp(R[i,j]-max_j R), zero elsewhere.

Both Aq and Mk are data-independent, so the heavy part is two streaming
matmuls over q and k per bh (memory-bound). Sharding: bh axis across 8
cores, 8 bh per core, zero communication.

The shipped v3 design (see DEFAULT_VARIANT comment for the measurements
behind it): q/k are cast f32->bf16 on the host (RN), halving the bytes the
device must stream through the per-core 435 GB/s SBUF DMA fabric:
8.39 MB/core -> ~19.3 us floor vs 38.5 us for f32. Weight precision is
preserved by factoring: the bulk of Aq/Mk is exactly-representable 0/1
bulk weights (bf16), Mk's own-bucket block gets a bf16 hi+lo Dekker-style
pair (both fused into one M=128 stationary operand so the lo pass costs no
extra matmuls), and the per-row fp32 scales s_i*c_j are applied to the tiny
[64,64] R product. End-to-end rel err 7.5e-3 (gate 2e-2), 4/4096 argmax
flips, identical to the CPU simulation of the same numerics.

PE layout per 32-row chunk r: stationary lhsT = weights [K=128, M=64|128],
moving rhs = data tile slice [K=128, N=nbh*64] spanning nbh heads, so one
LDWEIGHTS serves nbh heads; sq/sk land as [i|j, (bh,d)] in PSUM, get
PE-transposed per head-pair to [(b,d), i|j], and a K=64 matmul contracts d
to form R per head. Softmax + hard top-1 on DVE/ACT as in the f32 design.
"""

import numpy as np

BH, SEQ, DIM = 64, 4096, 64
NCORES = 8
BH_PER_CORE = BH // NCORES
GROUPS = BH_PER_CORE // 2  # 2 bh per group
FLTMAX = float(np.finfo(np.float32).max)

_CACHE = {}


def _constants():
    t = np.arange(SEQ, dtype=np.float64)
    i = np.arange(64, dtype=np.float64)[:, None]
    # Aq[i, t] = 1/(8*(64i+1)) for t <= 64i else 0   (includes the dim^-0.5 = 1/8)
    aq = np.where(t[None, :] <= 64 * i, 1.0 / (8.0 * (64 * i + 1.0)), 0.0)
    # Mk[j, t]: weight of k[t] in sk[j] = sum over bucket-j of cumavg
    inv = 1.0 / (t + 1.0)
    invb = inv.reshape(64, 64)
    suffix = np.cumsum(invb[:, ::-1], axis=1)[:, ::-1]  # suffix[j, s] = sum_{u>=s} 1/(64j+u+1)
    cj = invb.sum(axis=1)
    mk = np.zeros((64, SEQ))
    for j in range(64):
        mk[j, : 64 * j] = cj[j]
        mk[j, 64 * j : 64 * j + 64] = suffix[j]
    # SBUF weight layout [p, r, i] with t = 32p + r
    wq = aq.T.reshape(128, 32, 64).astype(np.float32)
    wk = mk.T.reshape(128, 32, 64).astype(np.float32)
    wq = np.ascontiguousarray(wq)
    wk = np.ascontiguousarray(wk)
    # additive causal mask on R[:, 1:]: masked where jc >= i
    maskadd = np.where(
        np.arange(64)[None, :] >= np.arange(64)[:, None], -FLTMAX, 0.0
    ).astype(np.float32)
    return wq, wk, maskadd


def _constants_v2():
    """Factored bf16 weights + fp32 scales.

    sq[i] = s_i * (Wq_hat^T q)[i],  Wq_hat[t,i] = 1 where t<=64i (exact in bf16)
    sk[j] = c_j * ((Wk_hi + Wk_lo)^T k)[j],  Wk_hat[t,j] = 1 for t<64j,
      suffix[j,s]/c_j in own bucket; lo = bf16 residual of hat (so the pair
      carries ~fp32 weight precision through two bf16 matmul passes).
    R[i,j] = (shq[i]·shk[j]) * SC[i,j],  SC = s_i*c_j, applied on the tiny
    [64,64] result so the coherent bf16 weight-scale error never enters.
    """
    from ml_dtypes import bfloat16 as bf

    t = np.arange(SEQ, dtype=np.float64)
    i = np.arange(64, dtype=np.float64)
    wq_hat = (t[:, None] <= 64.0 * i[None, :]).astype(np.float64)  # [t, i]
    inv = 1.0 / (t + 1.0)
    invb = inv.reshape(64, 64)
    suffix = np.cumsum(invb[:, ::-1], axis=1)[:, ::-1]  # [j, s]
    cj = invb.sum(axis=1)
    wk_hat = np.zeros((SEQ, 64))
    for j in range(64):
        wk_hat[: 64 * j, j] = 1.0
        wk_hat[64 * j : 64 * j + 64, j] = suffix[j] / cj[j]
    wk_hi = wk_hat.astype(bf)
    wk_lo = (wk_hat - wk_hi.astype(np.float64)).astype(bf)
    wq_sb = np.ascontiguousarray(wq_hat.astype(bf).reshape(128, 32, 64))
    wk_sb = np.ascontiguousarray(
        np.concatenate([wk_hi, wk_lo], axis=1).reshape(128, 32, 128)
    )
    s_i = 1.0 / (8.0 * (64.0 * i + 1.0))
    sc = (s_i[:, None] * cj[None, :]).astype(np.float32)  # [i, j]
    maskadd = np.where(
        np.arange(64)[None, :] >= np.arange(64)[:, None], -FLTMAX, 0.0
    ).astype(np.float32)
    ident = np.eye(64, dtype=np.float32)
    return wq_sb, wk_sb, sc, maskadd, ident


def _build_nc_v2(reps=1, dma_only=False):
    """bf16 kernel: f32->bf16 cast during DMA (SWDGE) halves SBUF-write bytes.

    PE restructured weights-stationary: per r-chunk one LDW + one matmul with
    the 2-bh data tile as the N=128 moving operand; k's hi|lo correction pair
    fused into one M=128 stationary (the lo pass rides free). sq/sk come out
    [i/j, (b,d)]; PE-transpose to [(b,d), i/j] for the d-contracting R matmul.
    """
    from contextlib import ExitStack

    import concourse.bacc as bacc
    import concourse.mybir as mybir
    import concourse.tile as tile

    f32 = mybir.dt.float32
    bf16 = mybir.dt.bfloat16
    wq_np, wk_np, sc_np, mask_np, id_np = _constants_v2()

    nc = bacc.Bacc(trn_type="TRN2")
    q = nc.dram_tensor("q", [BH_PER_CORE, SEQ, DIM], f32, kind="ExternalInput")
    k = nc.dram_tensor("k", [BH_PER_CORE, SEQ, DIM], f32, kind="ExternalInput")
    out = nc.dram_tensor("out", [BH_PER_CORE, 64, 65], f32, kind="ExternalOutput")
    wq_dram = nc.inline_tensor(wq_np, "wq_const")
    wk_dram = nc.inline_tensor(wk_np, "wk_const")
    sc_dram = nc.inline_tensor(sc_np, "sc_const")
    mask_dram = nc.inline_tensor(mask_np, "mask_const")
    id_dram = nc.inline_tensor(id_np, "id_const")

    q_ap, k_ap, out_ap = q.ap(), k.ap(), out.ap()

    with tile.TileContext(nc) as tc, ExitStack() as ctx:
        singles = ctx.enter_context(tc.tile_pool(name="singles", bufs=1))
        data = ctx.enter_context(tc.tile_pool(name="data", bufs=3))
        small = ctx.enter_context(tc.tile_pool(name="small", bufs=3))
        psum = ctx.enter_context(tc.tile_pool(name="psum", bufs=2, space="PSUM"))
        tpsum = ctx.enter_context(tc.tile_pool(name="tpsum", bufs=2, space="PSUM"))
        rpsum = ctx.enter_context(tc.tile_pool(name="rpsum", bufs=2, space="PSUM"))

        # Constants on the two HWDGE rings (data is on the SWDGE/gpsimd queue).
        wq_sb = singles.tile([128, 32, 64], bf16)
        wk_sb = singles.tile([128, 32, 128], bf16)
        sc_sb = singles.tile([64, 64], f32)
        mask_sb = singles.tile([64, 64], f32)
        id_sb = singles.tile([64, 64], f32)
        nc.sync.dma_start(wq_sb[:], wq_dram.ap())
        nc.scalar.dma_start(wk_sb[:], wk_dram.ap())
        nc.sync.dma_start(sc_sb[:], sc_dram.ap())
        nc.scalar.dma_start(mask_sb[:], mask_dram.ap())
        nc.sync.dma_start(id_sb[:], id_dram.ap())

        for rep_g in range(reps * GROUPS):
            g = rep_g % GROUPS
            qt = data.tile([128, 2, 32, 64], bf16, tag="qt")
            kt = data.tile([128, 2, 32, 64], bf16, tag="kt")
            for b in range(2):
                nc.gpsimd.dma_start(
                    qt[:, b], q_ap[2 * g + b].rearrange("(p r) d -> p r d", p=128)
                )
                nc.gpsimd.dma_start(
                    kt[:, b], k_ap[2 * g + b].rearrange("(p r) d -> p r d", p=128)
                )
            if dma_only:
                continue
            psq = psum.tile([64, 128], f32, tag="psq")
            psk = psum.tile([128, 128], f32, tag="psk")
            for r in range(32):
                nc.tensor.matmul(
                    psq[:], lhsT=wq_sb[:, r, :], rhs=qt[:, :, r, :],
                    start=(r == 0), stop=(r == 31),
                )
            for r in range(32):
                nc.tensor.matmul(
                    psk[:], lhsT=wk_sb[:, r, :], rhs=kt[:, :, r, :],
                    start=(r == 0), stop=(r == 31),
                )
            sqs = small.tile([64, 128], f32, tag="sqs")
            sks = small.tile([64, 128], f32, tag="sks")
            nc.vector.tensor_copy(sqs[:], psq[:])
            nc.vector.tensor_add(sks[:], psk[0:64, :], psk[64:128, :])
            pqT = tpsum.tile([128, 64], f32, tag="pqT")
            pkT = tpsum.tile([128, 64], f32, tag="pkT")
            nc.tensor.transpose(pqT[:], sqs[:], id_sb[:])
            nc.tensor.transpose(pkT[:], sks[:], id_sb[:])
            sqT = small.tile([128, 64], f32, tag="sqT")
            skT = small.tile([128, 64], f32, tag="skT")
            nc.vector.tensor_copy(sqT[:], pqT[:])
            nc.vector.tensor_copy(skT[:], pkT[:])
            for b in range(2):
                bh = 2 * g + b
                pr = rpsum.tile([64, 64], f32, tag="pr")
                nc.tensor.matmul(
                    pr[:],
                    lhsT=sqT[64 * b : 64 * b + 64, :],
                    rhs=skT[64 * b : 64 * b + 64, :],
                    start=True, stop=True,
                )
                rm = small.tile([64, 64], f32, tag="rm")
                nc.vector.tensor_mul(rm[:], pr[:], sc_sb[:])
                rf = small.tile([64, 65], f32, tag="rf")
                nc.vector.memset(rf[:, 0:1], 0.0)
                nc.vector.tensor_add(rf[:, 1:65], rm[:], mask_sb[:])
                m = small.tile([64, 1], f32, tag="m")
                nm = small.tile([64, 1], f32, tag="nm")
                s = small.tile([64, 1], f32, tag="s")
                rr = small.tile([64, 1], f32, tag="rr")
                nc.vector.reduce_max(m[:], rf[:], axis=mybir.AxisListType.X)
                nc.vector.tensor_scalar_mul(nm[:], m[:], -1.0)
                e = small.tile([64, 65], f32, tag="e")
                nc.scalar.activation(
                    e[:], rf[:], mybir.ActivationFunctionType.Exp,
                    bias=nm[:], scale=1.0, accum_out=s[:],
                )
                nc.vector.reciprocal(rr[:], s[:])
                o = small.tile([64, 65], f32, tag="o")
                nc.vector.tensor_scalar(
                    out=o[:], in0=rf[:], scalar1=m[:], scalar2=rr[:],
                    op0=mybir.AluOpType.is_equal, op1=mybir.AluOpType.mult,
                )
                nc.sync.dma_start(out_ap[bh], o[:])

    nc.compile()
    nc._kern_key = (reps, dma_only, "v2")
    return nc


def _build_nc_v3(reps=1, gpb=2, host_bf=True, dma_only=False):
    """Block-fused bf16 kernel. gpb = 2-bh groups per block: the moving
    matmul operand spans nbh=2*gpb heads (N = nbh*64 columns), so each
    32-chunk weight-stationary pass costs one LDWEIGHTS for nbh heads.

    host_bf=True: q/k arrive bf16 (host-cast), plain HWDGE loads.
    host_bf=False: q/k arrive f32, SWDGE (gpsimd) cast-DMA to bf16.
    """
    from contextlib import ExitStack

    import concourse.bacc as bacc
    import concourse.mybir as mybir
    import concourse.tile as tile

    f32 = mybir.dt.float32
    bf16 = mybir.dt.bfloat16
    wq_np, wk_np, sc_np, mask_np, id_np = _constants_v2()

    nbh = 2 * gpb
    ncol = nbh * 64
    nblk = BH_PER_CORE // nbh

    nc = bacc.Bacc(trn_type="TRN2")
    ddt = bf16 if host_bf else f32
    q = nc.dram_tensor("q", [BH_PER_CORE, SEQ, DIM], ddt, kind="ExternalInput")
    k = nc.dram_tensor("k", [BH_PER_CORE, SEQ, DIM], ddt, kind="ExternalInput")
    out = nc.dram_tensor("out", [BH_PER_CORE, 64, 65], f32, kind="ExternalOutput")
    wq_dram = nc.inline_tensor(wq_np, "wq_const")
    wk_dram = nc.inline_tensor(wk_np, "wk_const")
    sc_dram = nc.inline_tensor(sc_np, "sc_const")
    mask_dram = nc.inline_tensor(mask_np, "mask_const")
    id_dram = nc.inline_tensor(id_np, "id_const")

    q_ap, k_ap, out_ap = q.ap(), k.ap(), out.ap()

    with tile.TileContext(nc) as tc, ExitStack() as ctx:
        singles = ctx.enter_context(tc.tile_pool(name="singles", bufs=1))
        data = ctx.enter_context(
            tc.tile_pool(name="data", bufs=2 if gpb == 4 else 3)
        )
        small = ctx.enter_context(tc.tile_pool(name="small", bufs=3))
        psum = ctx.enter_context(tc.tile_pool(name="psum", bufs=2, space="PSUM"))
        tpsum = ctx.enter_context(tc.tile_pool(name="tpsum", bufs=2, space="PSUM"))
        rpsum = ctx.enter_context(tc.tile_pool(name="rpsum", bufs=2, space="PSUM"))

        wq_sb = singles.tile([128, 32, 64], bf16)
        wk_sb = singles.tile([128, 32, 128], bf16)
        sc_sb = singles.tile([64, 64], f32)
        mask_sb = singles.tile([64, 64], f32)
        id_sb = singles.tile([64, 64], f32)
        if host_bf:
            nc.gpsimd.dma_start(wq_sb[:], wq_dram.ap())
            nc.gpsimd.dma_start(wk_sb[:], wk_dram.ap())
            nc.gpsimd.dma_start(sc_sb[:], sc_dram.ap())
            nc.gpsimd.dma_start(mask_sb[:], mask_dram.ap())
            nc.gpsimd.dma_start(id_sb[:], id_dram.ap())
        else:
            nc.sync.dma_start(wq_sb[:], wq_dram.ap())
            nc.scalar.dma_start(wk_sb[:], wk_dram.ap())
            nc.sync.dma_start(sc_sb[:], sc_dram.ap())
            nc.scalar.dma_start(mask_sb[:], mask_dram.ap())
            nc.sync.dma_start(id_sb[:], id_dram.ap())

        for rep_b in range(reps * nblk):
            blk = rep_b % nblk
            qt = data.tile([128, nbh, 32, 64], bf16, tag="qt")
            kt = data.tile([128, nbh, 32, 64], bf16, tag="kt")
            src_q = q_ap[nbh * blk : nbh * (blk + 1)].rearrange(
                "b (p r) d -> p b r d", p=128
            )
            src_k = k_ap[nbh * blk : nbh * (blk + 1)].rearrange(
                "b (p r) d -> p b r d", p=128
            )
            if host_bf:
                nc.sync.dma_start(qt[:], src_q)
                nc.scalar.dma_start(kt[:], src_k)
            else:
                nc.gpsimd.dma_start(qt[:], src_q)
                nc.gpsimd.dma_start(kt[:], src_k)
            if dma_only:
                continue
            psq = psum.tile([64, ncol], f32, tag="psq")
            psk = psum.tile([128, ncol], f32, tag="psk")
            for r in range(32):
                nc.tensor.matmul(
                    psq[:], lhsT=wq_sb[:, r, :], rhs=qt[:, :, r, :],
                    start=(r == 0), stop=(r == 31),
                )
            for r in range(32):
                nc.tensor.matmul(
                    psk[:], lhsT=wk_sb[:, r, :], rhs=kt[:, :, r, :],
                    start=(r == 0), stop=(r == 31),
                )
            sqs = small.tile([64, ncol], f32, tag="sqs")
            sks = small.tile([64, ncol], f32, tag="sks")
            klo = small.tile([64, ncol], f32, tag="klo")
            nc.vector.tensor_copy(sqs[:], psq[:])
            nc.vector.tensor_copy(klo[:], psk[64:128, :])
            nc.vector.tensor_add(sks[:], psk[0:64, :], klo[:])
            for g2 in range(gpb):
                sl = slice(128 * g2, 128 * (g2 + 1))
                pT = tpsum.tile([128, 2, 64], f32, tag="pT")
                nc.tensor.transpose(pT[:, 0], sqs[:, sl], id_sb[:])
                nc.tensor.transpose(pT[:, 1], sks[:, sl], id_sb[:])
                sqT = small.tile([128, 64], f32, tag="sqT")
                skT = small.tile([128, 64], f32, tag="skT")
                nc.vector.tensor_copy(sqT[:], pT[:, 0])
                nc.vector.tensor_copy(skT[:], pT[:, 1])
                for b in range(2):
                    bh = nbh * blk + 2 * g2 + b
                    pr = rpsum.tile([64, 64], f32, tag="pr")
                    nc.tensor.matmul(
                        pr[:],
                        lhsT=sqT[64 * b : 64 * b + 64, :],
                        rhs=skT[64 * b : 64 * b + 64, :],
                        start=True, stop=True,
                    )
                    rm = small.tile([64, 64], f32, tag="rm")
                    nc.vector.tensor_mul(rm[:], pr[:], sc_sb[:])
                    rf = small.tile([64, 65], f32, tag="rf")
                    nc.vector.memset(rf[:, 0:1], 0.0)
                    nc.vector.tensor_add(rf[:, 1:65], rm[:], mask_sb[:])
                    m = small.tile([64, 1], f32, tag="m")
                    nm = small.tile([64, 1], f32, tag="nm")
                    s = small.tile([64, 1], f32, tag="s")
                    rr = small.tile([64, 1], f32, tag="rr")
                    nc.vector.reduce_max(m[:], rf[:], axis=mybir.AxisListType.X)
                    nc.vector.tensor_scalar_mul(nm[:], m[:], -1.0)
                    e = small.tile([64, 65], f32, tag="e")
                    nc.scalar.activation(
                        e[:], rf[:], mybir.ActivationFunctionType.Exp,
                        bias=nm[:], scale=1.0, accum_out=s[:],
                    )
                    nc.vector.reciprocal(rr[:], s[:])
                    o = small.tile([64, 65], f32, tag="o")
                    nc.vector.tensor_scalar(
                        out=o[:], in0=rf[:], scalar1=m[:], scalar2=rr[:],
                        op0=mybir.AluOpType.is_equal, op1=mybir.AluOpType.mult,
                    )
                    nc.sync.dma_start(out_ap[bh], o[:])

    nc.compile()
    nc._kern_key = (reps, dma_only, f"v3-{gpb}-{int(host_bf)}")
    return nc


def _build_nc(reps=1, dma_only=False, variant=0):
    if variant == "v2":
        return _build_nc_v2(reps, dma_only)
    if isinstance(variant, str) and variant.startswith("v3"):
        _, gpb, hb = variant.split("-")
        return _build_nc_v3(reps, gpb=int(gpb), host_bf=bool(int(hb)), dma_only=dma_only)
    """variant 0: all input DMAs on the SP HWDGE ring, 2MB each, bufs=2.
    variant 1: q on SP ring / k on ACT ring, per-bh 1MB DMAs, bufs=3.
    variant 2: variant 1 + skip q rows t in [4064, 4096) (partition 127):
      they are never used (sq[63] needs only t<=4032) and their Aq weight
      rows are zero, so q DMAs load 127 partitions and q matmuls contract
      K=127 — bit-identical output, 0.38% fewer HBM bytes."""
    from contextlib import ExitStack

    import concourse.bacc as bacc
    import concourse.mybir as mybir
    import concourse.tile as tile

    f32 = mybir.dt.float32
    wq_np, wk_np, mask_np = _constants()

    nc = bacc.Bacc(trn_type="TRN2")
    q = nc.dram_tensor("q", [BH_PER_CORE, SEQ, DIM], f32, kind="ExternalInput")
    k = nc.dram_tensor("k", [BH_PER_CORE, SEQ, DIM], f32, kind="ExternalInput")
    out = nc.dram_tensor("out", [BH_PER_CORE, 64, 65], f32, kind="ExternalOutput")
    wq_dram = nc.inline_tensor(wq_np, "wq_const")
    wk_dram = nc.inline_tensor(wk_np, "wk_const")
    mask_dram = nc.inline_tensor(mask_np, "mask_const")

    q_ap, k_ap, out_ap = q.ap(), k.ap(), out.ap()

    with tile.TileContext(nc) as tc, ExitStack() as ctx:
        singles = ctx.enter_context(tc.tile_pool(name="singles", bufs=1))
        data = ctx.enter_context(tc.tile_pool(name="data", bufs=3 if variant else 2))
        small = ctx.enter_context(tc.tile_pool(name="small", bufs=3))
        psum = ctx.enter_context(tc.tile_pool(name="psum", bufs=2, space="PSUM"))
        rpsum = ctx.enter_context(tc.tile_pool(name="rpsum", bufs=2, space="PSUM"))

        # Constants go on the SWDGE (gpsimd) queue so they don't serialize
        # ahead of the first data loads on the two HWDGE rings.
        wq_sb = singles.tile([128, 32, 64], f32)
        wk_sb = singles.tile([128, 32, 64], f32)
        mask_sb = singles.tile([64, 64], f32)
        nc.gpsimd.dma_start(wq_sb[:], wq_dram.ap())
        nc.gpsimd.dma_start(wk_sb[:], wk_dram.ap())
        nc.gpsimd.dma_start(mask_sb[:], mask_dram.ap())

        for rep_g in range(reps * GROUPS):
            g = rep_g % GROUPS
            qt = data.tile([128, 2, 32, 64], f32, tag="qt")
            kt = data.tile([128, 2, 32, 64], f32, tag="kt")
            if variant:
                qp = 127 if variant >= 2 else 128  # q partitions loaded/contracted
                for b in range(2):
                    nc.sync.dma_start(
                        qt[:qp, b],
                        q_ap[2 * g + b][: qp * 32].rearrange(
                            "(p r) d -> p r d", p=qp
                        ),
                    )
                    nc.scalar.dma_start(
                        kt[:, b],
                        k_ap[2 * g + b].rearrange("(p r) d -> p r d", p=128),
                    )
            else:
                nc.sync.dma_start(
                    qt[:],
                    q_ap[2 * g : 2 * g + 2].rearrange("b (p r) d -> p b r d", p=128),
                )
                nc.sync.dma_start(
                    kt[:],
                    k_ap[2 * g : 2 * g + 2].rearrange("b (p r) d -> p b r d", p=128),
                )
            if dma_only:
                continue
            psq = psum.tile([128, 64], f32, tag="psq")
            psk = psum.tile([128, 64], f32, tag="psk")
            # Stationary (weights) APs must be 2D [K, M] for walrus, so one
            # matmul per bh half: out partitions 64b..64b+64 of the PSUM tile.
            qp = 127 if variant >= 2 else 128
            for b in range(2):
                for r in range(32):
                    nc.tensor.matmul(
                        psq[64 * b : 64 * b + 64, :],
                        lhsT=qt[:qp, b, r, :], rhs=wq_sb[:qp, r, :],
                        start=(r == 0), stop=(r == 31),
                    )
            for b in range(2):
                for r in range(32):
                    nc.tensor.matmul(
                        psk[64 * b : 64 * b + 64, :],
                        lhsT=kt[:, b, r, :], rhs=wk_sb[:, r, :],
                        start=(r == 0), stop=(r == 31),
                    )
            sq_sb = small.tile([128, 64], f32, tag="sq")
            sk_sb = small.tile([128, 64], f32, tag="sk")
            nc.vector.tensor_copy(sq_sb[:], psq[:])
            nc.vector.tensor_copy(sk_sb[:], psk[:])
            for b in range(2):
                bh = 2 * g + b
                pr = rpsum.tile([64, 64], f32, tag="pr")
                nc.tensor.matmul(
                    pr[:],
                    lhsT=sq_sb[64 * b : 64 * b + 64, :],
                    rhs=sk_sb[64 * b : 64 * b + 64, :],
                    start=True, stop=True,
                )
                rf = small.tile([64, 65], f32, tag="rf")
                nc.vector.memset(rf[:, 0:1], 0.0)
                nc.vector.tensor_add(rf[:, 1:65], pr[:], mask_sb[:])
                m = small.tile([64, 1], f32, tag="m")
                nm = small.tile([64, 1], f32, tag="nm")
                s = small.tile([64, 1], f32, tag="s")
                rr = small.tile([64, 1], f32, tag="rr")
                nc.vector.reduce_max(m[:], rf[:], axis=mybir.AxisListType.X)
                nc.vector.tensor_scalar_mul(nm[:], m[:], -1.0)
                e = small.tile([64, 65], f32, tag="e")
                nc.scalar.activation(
                    e[:], rf[:], mybir.ActivationFunctionType.Exp,
                    bias=nm[:], scale=1.0, accum_out=s[:],
                )
                nc.vector.reciprocal(rr[:], s[:])
                o = small.tile([64, 65], f32, tag="o")
                nc.vector.tensor_scalar(
                    out=o[:], in0=rf[:], scalar1=m[:], scalar2=rr[:],
                    op0=mybir.AluOpType.is_equal, op1=mybir.AluOpType.mult,
                )
                nc.sync.dma_start(out_ap[bh], o[:])

    nc.compile()
    nc._kern_key = (reps, dma_only, variant)
    return nc


def _get_nc(reps=1, dma_only=False, variant=0):
    key = ("nc", reps, dma_only, variant)
    if key not in _CACHE:
        _CACHE[key] = _build_nc(reps, dma_only, variant)
    return _CACHE[key]


def _make_runner(nc):
    """Persistent jit(shard_map) callable over the 8 cores for one Bass module.

    One function object per nc so jax.jit's cache is reused across calls
    (run_bass_kernel_spmd re-traces on every invocation)."""
    import jax
    from jax.sharding import Mesh, PartitionSpec
    from jax.experimental.shard_map import shard_map

    import concourse.mybir as mybir
    from concourse.bass2jax import (
        _bass_exec_p,
        install_neuronx_cc_hook,
        partition_id_tensor,
    )

    install_neuronx_cc_hook()

    partition_name = nc.partition_id_tensor.name if nc.partition_id_tensor else None
    in_names, out_names, out_avals, zero_shapes = [], [], [], []
    for alloc in nc.m.functions[0].allocations:
        if not isinstance(alloc, mybir.MemoryLocationSet):
            continue
        name = alloc.memorylocations[0].name
        if alloc.kind == "ExternalInput":
            if name != partition_name:
                in_names.append(name)
        elif alloc.kind == "ExternalOutput":
            out_names.append(name)
            shape = tuple(alloc.tensor_shape)
            dtype = mybir.dt.np(alloc.dtype)
            out_avals.append(jax.core.ShapedArray(shape, dtype))
            zero_shapes.append((shape, dtype))
    n_params = len(in_names)
    n_outs = len(out_avals)
    all_in_names = tuple(
        in_names + out_names + ([partition_name] if partition_name else [])
    )

    def _body(*args):
        operands = list(args)
        if partition_name is not None:
            operands.append(partition_id_tensor())
        return tuple(
            _bass_exec_p.bind(
                *operands,
                out_avals=tuple(out_avals),
                in_names=all_in_names,
                out_names=tuple(out_names),
                lowering_input_output_aliases=(),
                sim_require_finite=True,
                sim_require_nnan=True,
                nc=nc,
            )
        )

    devices = jax.devices()[:NCORES]
    mesh = Mesh(np.asarray(devices), ("core",))
    _CACHE[("runner_mesh",) + getattr(nc, "_kern_key", (1, False, 0))] = mesh
    fn = jax.jit(
        shard_map(
            _body,
            mesh=mesh,
            in_specs=(PartitionSpec("core"),) * (n_params + n_outs),
            out_specs=(PartitionSpec("core"),) * n_outs,
            check_rep=False,
        ),
        donate_argnums=tuple(range(n_params, n_params + n_outs)),
        keep_unused=True,
    )

    name_to_idx = {n: i for i, n in enumerate(in_names)}
    out_idx = out_names.index("out")

    def run(q, k):
        import jax as _jax

        ins = [None] * n_params
        ins[name_to_idx["q"]] = q
        ins[name_to_idx["k"]] = k
        zeros = [
            np.zeros((NCORES * s[0], *s[1:]), dt) for (s, dt) in zero_shapes
        ]
        outs = fn(*ins, *zeros)
        _jax.block_until_ready(outs)
        return np.asarray(outs[out_idx]).reshape(BH, 64, 65)

    return run


def _get_runner(reps=1, dma_only=False, variant=0):
    key = ("runner", reps, dma_only, variant)
    if key not in _CACHE:
        _CACHE[key] = _make_runner(_get_nc(reps, dma_only, variant))
    return _CACHE[key]


def _prep(q, k):
    q = np.ascontiguousarray(np.asarray(q), dtype=np.float32)
    k = np.ascontiguousarray(np.asarray(k), dtype=np.float32)
    return q, k


def _prep_variant(variant, q, k):
    q, k = _prep(q, k)
    if isinstance(variant, str) and variant.startswith("v3") and variant.endswith("-1"):
        from ml_dtypes import bfloat16

        q = q.astype(bfloat16)
        k = k.astype(bfloat16)
    return q, k


# Default shipped configuration: v3 with gpb=2 (4 bh per block), host-side
# f32->bf16 cast (round-to-nearest), dual HWDGE rings (q on SP, k on ACT),
# 2MB bf16 DMAs. Rationale (measured, reps=1001 A/B):
#   - f32 loads saturate the 16-port SBUF AXI fabric at ~435 GB/s ->
#     16.78 MB/core = 38.5 us. That is a hard device floor for f32.
#   - SWDGE (gpsimd) f32->bf16 cast-DMA is SOURCE-byte-bound (~51-57 us
#     at any DMA size) - the SDMA datapath processes f32 source bytes, so
#     on-device casting wins nothing.
#   - Host-cast bf16 + HWDGE measured ~17-20 us/rep (dma-only), matching
#     the 8.39 MB/435 GB/s port floor.
# Accuracy: bf16 data with factored effectively-fp32 weights sims at
# rel=7.5e-3 (4/4096 argmax flips), well under the 2e-2 gate.
DEFAULT_VARIANT = "v3-2-1"


def _run_spmd(q, k, trace=False, **kwargs):
    q, k = _prep_variant(DEFAULT_VARIANT, q, k)
    out = _get_runner(1, variant=DEFAULT_VARIANT)(q, k)
    return out, None


def kernel(q, k, topk=1):
    q, k = _prep_variant(DEFAULT_VARIANT, q, k)
    return _get_runner(1, variant=DEFAULT_VARIANT)(q, k)

